# revision 5
# baseline (speedup 1.0000x reference)
"""Trainium2 Bass kernel for nn_Columbina_Model (2-layer hetero GAT).

Strategy: dst-shard gene nodes across 8 cores (12500 each, padded to 12544).
Host sorts each core's edges by destination, groups them into 128-edge chunks
per 128-node destination tile. On device: encoders + BN stats (AllReduce) are
computed per-shard; BN is folded into the GAT linear weights; each core builds
its shard of a gather table [h1|a_s] and tables are AllGathered. The edge
phase gathers 288B table rows by src via indirect DMA, computes attention
weights (leaky-relu + exp), and segment-sums messages into per-node-tile PSUM
accumulators with one-hot matmuls — no indirect scatter needed. Softmax
normalization, head-mean, bias and residual run per node tile.
"""
import json
import sys
import types

import numpy as np
import ml_dtypes

H, C = 4, 32
NG = ND = 100000
M = 8
RN = 12500
RPAD = 12544            # 98 * 128
NTILES = RPAD // 128
NT = RPAD * M           # 100352 table rows
DUMMY = NT              # dummy table row (a_s = -1e9, h = 0)
TROWS = NT + 8          # table allocation rows
ROWW = 144              # table row width (bf16) = 288B; cols 0..128 h, 128..132 a_s
BN_EPS = 1e-5

bf16 = ml_dtypes.bfloat16

LAST_RESULT = None

# --------------------------------------------------------------------------
# runtime patches (this container lacks antenv.axon_hooks; walrus rejects >1
# sync wait per instruction)
# --------------------------------------------------------------------------


def _install_patches():
    if "antenv.axon_hooks" not in sys.modules:
        try:
            import antenv
            from trn_agent_boot.trn_boot import _ntff_profile_via_ctypes

            m = types.ModuleType("antenv.axon_hooks")
            _state = {"hook": _ntff_profile_via_ctypes("/opt/axon/libaxon_pjrt.so")}
            m.get_axon_ntff_profile_hook = lambda: _state["hook"]
            m.set_axon_ntff_profile_hook = lambda h: _state.__setitem__("hook", h)
            sys.modules["antenv.axon_hooks"] = m
            antenv.axon_hooks = m
        except Exception:
            pass

    import concourse.tile as tile
    from concourse.vector_clock import ScopedClock

    if not getattr(tile.TileContext, "_drain_patched", False):
        def _drain_and_barrier(self, tick_clock, wait_clock):
            gc = tick_clock.global_clock
            for proc in range(len(gc)):
                tick = gc[proc]
                if tick > 0:
                    partial = ScopedClock()
                    partial.require_at_least(None, proc, tick)
                    nop = self.nc.sync.nop(nofuse=True)
                    wait_clock.add_sem_waits(nop.ins, partial)
            self.nc.sync.drain()
            self.nc.all_engine_barrier()
            assert self.sems is not None
            popped = self.nc._tile_sem_poison_stack.pop()
            assert popped is self._sem_poison
            self.nc.clear_and_free_semaphores(list(self.sems.allocated().values()))
            self.nc.all_engine_barrier()

        tile.TileContext._drain_and_barrier = _drain_and_barrier
        tile.TileContext._drain_patched = True

    import concourse.bass_utils as bu
    import concourse.bass2jax as b2j

    if not getattr(bu, "_compile_patched", False):
        def _split(bir_json):
            d = json.loads(bir_json)
            for f in d.get("functions", []):
                for bb in f.get("blocks", []):
                    new_insts = []
                    for inst in bb["instructions"]:
                        si = inst.get("sync_info")
                        ow = (si or {}).get("on_wait") or []
                        if len(ow) > 1:
                            for i, w in enumerate(ow[:-1]):
                                new_insts.append({
                                    "debug": inst.get("debug", 0),
                                    "engine": inst["engine"],
                                    "ins": [], "outs": [],
                                    "name": f"{inst['name']}-w{i}",
                                    "opcode": "NoOp",
                                    "sync_info": {"on_update": [],
                                                  "on_wait": [w]},
                                })
                            si["on_wait"] = ow[-1:]
                        new_insts.append(inst)
                    bb["instructions"] = new_insts
            return json.dumps(d).encode()

        orig = bu.compile_bir_kernel

        def compile_bir_kernel(bir_json, tmpdir, neff_name="file.neff"):
            return orig(_split(bir_json), tmpdir, neff_name)

        bu.compile_bir_kernel = compile_bir_kernel
        b2j.compile_bir_kernel = compile_bir_kernel
        bu._compile_patched = True


# --------------------------------------------------------------------------
# host-side edge preprocessing
# --------------------------------------------------------------------------


def _fold_att(W, att):
    # W [32, 128], att [H, C] -> [32, H]
    return np.einsum('jhc,hc->jh', np.asarray(W).reshape(32, H, C), np.asarray(att))


def _remap(v):
    return (v // RN) * RPAD + (v % RN)


def _prep_edges(src, dst):
    """Per-core chunk arrays. Returns (src_idx [M,NCH,128] int32,
    seg [M,NCH,128] f32, tile_of_chunk [NCH] int, chunks_per_tile [NTILES])."""
    src = np.asarray(src).astype(np.int64)
    dst = np.asarray(dst).astype(np.int64)
    per_core = []
    counts = np.zeros((M, NTILES), np.int64)
    for k in range(M):
        m = (dst >= k * RN) & (dst < (k + 1) * RN)
        s, d = src[m], dst[m] - k * RN
        order = np.argsort(d, kind='stable')
        s, d = s[order], d[order]
        tile_id = d >> 7
        counts[k] = np.bincount(tile_id, minlength=NTILES)
        per_core.append((s, d, tile_id))
    cpt = np.maximum(1, (counts.max(axis=0) + 127) // 128)  # chunks per tile
    nch = int(cpt.sum())
    chunk_base = np.concatenate([[0], np.cumsum(cpt)])[:-1]  # per tile
    tile_of_chunk = np.repeat(np.arange(NTILES), cpt)
    src_idx = np.full((M, nch, 128), DUMMY, np.int32)
    seg = np.zeros((M, nch, 128), np.float32)
    for k in range(M):
        s, d, tile_id = per_core[k]
        tile_start = np.concatenate([[0], np.cumsum(counts[k])])[:-1]
        pos = np.arange(len(s)) - tile_start[tile_id]
        gchunk = chunk_base[tile_id] + (pos >> 7)
        slot = pos & 127
        src_idx[k, gchunk, slot] = _remap(s)
        seg[k, gchunk, slot] = (d & 127).astype(np.float32)
    return src_idx, seg, tile_of_chunk, cpt


def _wstack(*mats):
    return np.concatenate([np.asarray(m, np.float32) for m in mats], axis=1)


# --------------------------------------------------------------------------
# device kernel builder
# --------------------------------------------------------------------------


def _build(nch1, tiles1, starts1, stops1, nch2, tiles2, starts2, stops2):
    import concourse.bass as bass
    import concourse.mybir as mybir
    import concourse.tile as tile
    from concourse import bacc
    from concourse.bass import AP, IndirectOffsetOnAxis
    from concourse.masks import make_identity

    dt = mybir.dt
    nc = bacc.Bacc()

    def par(name, shape, dty, out=False):
        return nc.declare_dram_parameter(name, shape, dty, isOutput=out)

    xg = par("xg", [128, RPAD], dt.bfloat16)        # transposed gene shard
    xd = par("xd", [96, RPAD], dt.bfloat16)         # transposed drug shard
    wg = par("wg", [128, 32], dt.bfloat16)
    wd = par("wd", [96, 32], dt.bfloat16)
    bgv = par("bgv", [32, 1], dt.float32)           # encoder biases (per channel)
    bdv = par("bdv", [32, 1], dt.float32)
    statc = par("statc", [32, 4], dt.float32)       # pad-col stat corrections
    gamb = par("gamb", [32, 4], dt.float32)         # gamma_g, beta_g, gamma_d, beta_d
    w1c = par("w1c", [32, 168], dt.float32)         # [W1|A_s1|A_d1|I] combined gene
    w2c = par("w2c", [32, 132], dt.float32)         # [W2|A_s2] drug
    a2d = par("a2d", [32, 4], dt.bfloat16)          # fold(W2, att_d2), unscaled
    bias1r = par("bias1r", [128, 32], dt.float32)   # bias1 replicated
    bias2r = par("bias2r", [128, 32], dt.float32)
    iotap = par("iotap", [128, 128], dt.float32)
    s1i = par("s1i", [128, nch1], dt.int32)
    s1g = par("s1g", [128, nch1], dt.float32)
    s2i = par("s2i", [128, nch2], dt.int32)
    s2g = par("s2g", [128, nch2], dt.float32)
    outp = par("out", [RPAD, 32], dt.float32, out=True)

    with tile.TileContext(nc) as tc:
        with (
            tc.tile_pool(name="const", bufs=1) as cp,
            tc.tile_pool(name="enc", bufs=2) as ep,
            tc.tile_pool(name="work", bufs=4) as wp,
            tc.tile_pool(name="dram", bufs=1, space="DRAM") as dp,
        ):
            # ---------------- constants / params to SBUF ----------------
            t_wg = cp.tile([128, 32], dt.bfloat16)
            t_wd = cp.tile([96, 32], dt.bfloat16)
            t_bg = cp.tile([32, 1], dt.float32)
            t_bd = cp.tile([32, 1], dt.float32)
            t_statc = cp.tile([32, 4], dt.float32)
            t_gamb = cp.tile([32, 4], dt.float32)
            t_w1c = cp.tile([32, 168], dt.float32)
            t_w2c = cp.tile([32, 132], dt.float32)
            t_a2d = cp.tile([32, 4], dt.bfloat16)
            t_b1 = cp.tile([128, 32], dt.float32)
            t_b2 = cp.tile([128, 32], dt.float32)
            t_iota = cp.tile([128, 128], dt.float32)
            t_s1i = cp.tile([128, nch1], dt.int32)
            t_s1g = cp.tile([128, nch1], dt.float32)
            t_s2i = cp.tile([128, nch2], dt.int32)
            t_s2g = cp.tile([128, nch2], dt.float32)
            ident = cp.tile([128, 128], dt.bfloat16)
            make_identity(nc, ident[:])
            for t, p in [(t_wg, wg), (t_wd, wd), (t_bg, bgv), (t_bd, bdv),
                         (t_statc, statc), (t_gamb, gamb), (t_w1c, w1c),
                         (t_w2c, w2c), (t_a2d, a2d), (t_b1, bias1r),
                         (t_b2, bias2r), (t_iota, iotap), (t_s1i, s1i),
                         (t_s1g, s1g), (t_s2i, s2i), (t_s2g, s2g)]:
                nc.sync.dma_start(out=t[:], in_=p[:])
            t_xg = cp.tile([128, RPAD], dt.bfloat16)
            t_xd = cp.tile([96, RPAD], dt.bfloat16)
            nc.sync.dma_start(out=t_xg[:], in_=xg[:])
            nc.sync.dma_start(out=t_xd[:], in_=xd[:])

            pp = None  # set per-phase below
            # DRAM internals
            table1 = dp.tile([TROWS, ROWW], dt.bfloat16)
            table2 = dp.tile([TROWS, ROWW], dt.bfloat16)
            own1 = dp.tile([RPAD, ROWW], dt.bfloat16)
            own2 = dp.tile([RPAD, ROWW], dt.bfloat16)
            st_in = dp.tile([32, 4], dt.float32)
            st_out = dp.tile([32, 4], dt.float32)

            enc_psum = tc.tile_pool(name="ps_enc", bufs=2, space="PSUM")
            pp = enc_psum.__enter__()
            # ---------------- encoders: y_T = relu(W^T x_T + b) ---------
            def encode(xT, wT, bT, npart, tag):
                yT = cp.tile([33, RPAD], dt.bfloat16, tag=f"y{tag}")
                st = ep.tile([32, 50], dt.float32, tag=f"st{tag}")
                for i in range(25):
                    lo = i * 512
                    w = min(512, RPAD - lo)
                    ps = pp.tile([32, 512], dt.float32, space="PSUM", tag="encp")
                    nc.tensor.matmul(ps[:, :w], lhsT=wT[:npart, :],
                                     rhs=xT[:npart, lo:lo + w],
                                     start=True, stop=True)
                    nc.scalar.activation(
                        yT[0:32, lo:lo + w], ps[:, :w],
                        mybir.ActivationFunctionType.Relu, bias=bT[:],
                        accum_out=st[:, i:i + 1])
                    sq = ep.tile([32, 512], dt.float32, tag=f"sq{tag}")
                    nc.scalar.activation(
                        sq[:, :w], yT[0:32, lo:lo + w],
                        mybir.ActivationFunctionType.Square,
                        accum_out=st[:, 25 + i:26 + i])
                nc.vector.memset(yT[32:33, :], 1.0)
                s1 = ep.tile([32, 2], dt.float32, tag=f"s1{tag}")
                nc.vector.tensor_reduce(s1[:, 0:1], st[:, 0:25],
                                        mybir.AxisListType.X, mybir.AluOpType.add)
                nc.vector.tensor_reduce(s1[:, 1:2], st[:, 25:50],
                                        mybir.AxisListType.X, mybir.AluOpType.add)
                return yT, s1

            ygT, sg = encode(t_xg, t_wg, t_bg, 128, "g")
            ydT, sd = encode(t_xd, t_wd, t_bd, 96, "d")

            # stats allreduce: [32,4] = [sum_g, sq_g, sum_d, sq_d] - corrections
            packed = ep.tile([32, 4], dt.float32, tag="pk")
            nc.vector.tensor_copy(out=packed[:, 0:2], in_=sg[:])
            nc.vector.tensor_copy(out=packed[:, 2:4], in_=sd[:])
            nc.vector.tensor_sub(out=packed[:], in0=packed[:], in1=t_statc[:])
            nc.sync.dma_start(out=st_in[:], in_=packed[:])
            nc.gpsimd.collective_compute(
                "AllReduce", mybir.AluOpType.add,
                replica_groups=[list(range(M))],
                ins=[st_in.opt()], outs=[st_out.opt()])
            tstat = ep.tile([32, 4], dt.float32, tag="ts")
            nc.gpsimd.dma_start(out=tstat[:], in_=st_out[:])

            # BN fold: s = gamma/sqrt(var+eps), t = beta - mu*s  (cols: g, d)
            sfold = ep.tile([32, 2], dt.float32, tag="sf")
            tfold = ep.tile([32, 2], dt.float32, tag="tf")
            mu = ep.tile([32, 2], dt.float32, tag="mu")
            var = ep.tile([32, 2], dt.float32, tag="va")
            tmp = ep.tile([32, 2], dt.float32, tag="tm")
            nc.vector.tensor_scalar(out=mu[:, 0:1], in0=tstat[:, 0:1],
                                    scalar1=1.0 / NG, scalar2=None,
                                    op0=mybir.AluOpType.mult)
            nc.vector.tensor_scalar(out=mu[:, 1:2], in0=tstat[:, 2:3],
                                    scalar1=1.0 / ND, scalar2=None,
                                    op0=mybir.AluOpType.mult)
            nc.vector.tensor_scalar(out=var[:, 0:1], in0=tstat[:, 1:2],
                                    scalar1=1.0 / NG, scalar2=None,
                                    op0=mybir.AluOpType.mult)
            nc.vector.tensor_scalar(out=var[:, 1:2], in0=tstat[:, 3:4],
                                    scalar1=1.0 / ND, scalar2=None,
                                    op0=mybir.AluOpType.mult)
            nc.vector.tensor_mul(out=tmp[:], in0=mu[:], in1=mu[:])
            nc.vector.tensor_sub(out=var[:], in0=var[:], in1=tmp[:])
            nc.vector.tensor_scalar(out=var[:], in0=var[:], scalar1=BN_EPS,
                                    scalar2=None, op0=mybir.AluOpType.add)
            nc.scalar.activation(tmp[:], var[:], mybir.ActivationFunctionType.Sqrt)
            nc.vector.reciprocal(out=tmp[:], in_=tmp[:])
            nc.vector.tensor_mul(out=sfold[:, 0:1], in0=t_gamb[:, 0:1],
                                 in1=tmp[:, 0:1])
            nc.vector.tensor_mul(out=sfold[:, 1:2], in0=t_gamb[:, 2:3],
                                 in1=tmp[:, 1:2])
            nc.vector.tensor_mul(out=tfold[:], in0=mu[:], in1=sfold[:])
            nc.vector.tensor_sub(out=tfold[:, 0:1], in0=t_gamb[:, 1:2],
                                 in1=tfold[:, 0:1])
            nc.vector.tensor_sub(out=tfold[:, 1:2], in0=t_gamb[:, 3:4],
                                 in1=tfold[:, 1:2])

            # W augmentation.
            # gene: waug1 [33, 168] = [s_g*[W1|A_s1|A_d1|I] ; t_g@[...]]
            #   cols 0..132 table row, 132..136 a_d1, 136..168 g rows
            def build_waug(wcomb, ncols, sf_col, tf_col, tag):
                wa = cp.tile([33, ncols], dt.bfloat16, tag=f"wa{tag}")
                scaled = ep.tile([32, ncols], dt.float32, tag=f"sc{tag}")
                nc.vector.tensor_tensor(
                    out=scaled[:], in0=wcomb[:, :ncols],
                    in1=sf_col.to_broadcast([32, ncols]),
                    op=mybir.AluOpType.mult)
                nc.vector.tensor_copy(out=wa[0:32, :], in_=scaled[:])
                tr = pp.tile([1, ncols], dt.float32, space="PSUM", tag="warow")
                tscaled = ep.tile([32, 1], dt.float32, tag=f"tsc{tag}")
                nc.vector.tensor_copy(out=tscaled[:], in_=tf_col)
                nc.tensor.matmul(tr[:], lhsT=tscaled[:], rhs=wcomb[:, :ncols],
                                 start=True, stop=True)
                nc.vector.tensor_copy(out=wa[32:33, :], in_=tr[:])
                return wa

            waug1 = build_waug(t_w1c, 168, sfold[:, 0:1], tfold[:, 0:1], "1")
            waug2 = build_waug(t_w2c, 132, sfold[:, 1:2], tfold[:, 1:2], "2")

            # ---------------- own table builds ----------------
            a_d1 = cp.tile([128, NTILES * 4], dt.bfloat16)
            a_d2 = cp.tile([128, NTILES * 4], dt.bfloat16)
            g_own = cp.tile([128, NTILES * 32], dt.float32)

            for j in range(NTILES):
                ps = pp.tile([128, 168], dt.float32, space="PSUM", tag="tb")
                nc.tensor.matmul(ps[:], lhsT=ygT[:, bass.ts(j, 128)],
                                 rhs=waug1[:], start=True, stop=True)
                row = wp.tile([128, ROWW], dt.bfloat16, tag="tbr")
                nc.vector.tensor_copy(out=row[:, 0:132], in_=ps[:, 0:132])
                nc.vector.memset(row[:, 132:ROWW], 0.0)
                nc.sync.dma_start(out=own1[bass.ts(j, 128), :], in_=row[:])
                nc.vector.tensor_copy(out=a_d1[:, bass.ts(j, 4)],
                                      in_=ps[:, 132:136])
                nc.vector.tensor_copy(out=g_own[:, bass.ts(j, 32)],
                                      in_=ps[:, 136:168])
            for j in range(NTILES):
                ps = pp.tile([128, 132], dt.float32, space="PSUM", tag="tb2")
                nc.tensor.matmul(ps[:], lhsT=ydT[:, bass.ts(j, 128)],
                                 rhs=waug2[:], start=True, stop=True)
                row = wp.tile([128, ROWW], dt.bfloat16, tag="tbr2")
                nc.vector.tensor_copy(out=row[:, 0:132], in_=ps[:])
                nc.vector.memset(row[:, 132:ROWW], 0.0)
                nc.sync.dma_start(out=own2[bass.ts(j, 128), :], in_=row[:])

            # dummy rows (a_s = -1e9 so exp underflows to 0)
            drow = wp.tile([8, ROWW], dt.bfloat16, tag="drow")
            nc.vector.memset(drow[:], 0.0)
            nc.vector.memset(drow[:, 128:132], -1e9)
            nc.sync.dma_start(out=table1[NT:TROWS, :], in_=drow[:])
            nc.sync.dma_start(out=table2[NT:TROWS, :], in_=drow[:])

            enc_psum.__exit__(None, None, None)
            edge_psum = tc.tile_pool(name="ps_edge", bufs=2, space="PSUM")
            pp = edge_psum.__enter__()
            # AllGather tables
            nc.gpsimd.collective_compute(
                "AllGather", mybir.AluOpType.bypass,
                replica_groups=[list(range(M))],
                ins=[own1.opt()], outs=[table1[0:NT, :]])
            nc.gpsimd.collective_compute(
                "AllGather", mybir.AluOpType.bypass,
                replica_groups=[list(range(M))],
                ins=[own2.opt()], outs=[table2[0:NT, :]])

            # ---------------- edge phase ----------------
            def apv(t, part, dims, offset=0):
                a = t[:]
                return AP(a.tensor, a.offset + offset,
                          [(a.ap[0][0], part)] + list(dims))

            def edge_phase(nch, tiles, starts, stops, table, t_si, t_sg, a_d,
                           tag):
                """Returns list of per-tile psum handles consumed inline by
                caller via callback; instead accumulate and yield evac tiles"""
                evacs = {}
                ps = None
                for c in range(nch):
                    g = wp.tile([128, ROWW], dt.bfloat16, tag=f"g{tag}")
                    nc.gpsimd.indirect_dma_start(
                        out=g[:], out_offset=None, in_=table[:],
                        in_offset=IndirectOffsetOnAxis(
                            ap=t_si[:, c:c + 1], axis=0))
                    P = wp.tile([128, 128], dt.bfloat16, tag=f"P{tag}")
                    nc.vector.tensor_tensor(
                        out=P[:], in0=t_sg[:, c:c + 1].to_broadcast([128, 128]),
                        in1=t_iota[:], op=mybir.AluOpType.is_equal)
                    ptp = pp.tile([128, 128], dt.bfloat16, space="PSUM",
                                  tag="ptp", bufs=1)
                    nc.tensor.transpose(out=ptp[:], in_=P[:], identity=ident[:])
                    PT = wp.tile([128, 128], dt.bfloat16, tag=f"PT{tag}")
                    nc.vector.tensor_copy(out=PT[:], in_=ptp[:])
                    pa = pp.tile([128, 4], dt.float32, space="PSUM",
                                 tag="pa", bufs=1)
                    nc.tensor.matmul(
                        pa[:], lhsT=PT[:],
                        rhs=a_d[:, bass.ts(int(tiles[c]), 4)],
                        start=True, stop=True)
                    alpha = wp.tile([128, 4], dt.float32, tag=f"al{tag}")
                    nc.vector.tensor_tensor(out=alpha[:], in0=g[:, 128:132],
                                            in1=pa[:], op=mybir.AluOpType.add)
                    lr = wp.tile([128, 4], dt.float32, tag=f"lr{tag}")
                    nc.vector.scalar_tensor_tensor(
                        out=lr[:], in0=alpha[:], scalar=0.2,
                        in1=alpha[:], op0=mybir.AluOpType.mult,
                        op1=mybir.AluOpType.max)
                    rhs = wp.tile([128, 132], dt.bfloat16, tag=f"rh{tag}")
                    nc.scalar.activation(rhs[:, 128:132], lr[:],
                                         mybir.ActivationFunctionType.Exp)
                    nc.vector.tensor_tensor(
                        out=apv(rhs, 128, [(32, 4), (1, 32)]),
                        in0=apv(g, 128, [(32, 4), (1, 32)]),
                        in1=apv(rhs, 128, [(1, 4), (0, 32)], offset=128),
                        op=mybir.AluOpType.mult)
                    if starts[c]:
                        ps = pp.tile([128, 132], dt.float32, space="PSUM",
                                     tag="ac", bufs=2)
                    nc.tensor.matmul(ps[:], lhsT=P[:], rhs=rhs[:],
                                     start=bool(starts[c]),
                                     stop=bool(stops[c]))
                    if stops[c]:
                        ev = wp.tile([128, 132], dt.float32, tag=f"ev{tag}")
                        nc.vector.tensor_copy(out=ev[:], in_=ps[:])
                        evacs[tiles[c]] = ev
                        yield int(tiles[c]), ev

            def pass2(ev, tile_j, bias_t, resid_tile, outbuf):
                # z = mean_h(num_h / (den_h + eps)) + bias + resid
                den = wp.tile([128, 4], dt.float32, tag="den")
                nc.vector.tensor_scalar(out=den[:], in0=ev[:, 128:132],
                                        scalar1=1e-16, scalar2=None,
                                        op0=mybir.AluOpType.add)
                nc.vector.reciprocal(out=den[:], in_=den[:])
                nc.vector.tensor_scalar(out=den[:], in0=den[:],
                                        scalar1=1.0 / H, scalar2=None,
                                        op0=mybir.AluOpType.mult)
                num = wp.tile([128, 128], dt.float32, tag="num")
                nc.vector.tensor_tensor(
                    out=apv(num, 128, [(32, 4), (1, 32)]),
                    in0=apv(ev, 128, [(32, 4), (1, 32)]),
                    in1=apv(den, 128, [(1, 4), (0, 32)]),
                    op=mybir.AluOpType.mult)
                z = wp.tile([128, 32], dt.float32, tag="z")
                nc.vector.tensor_reduce(
                    z[:], apv(num, 128, [(1, 32), (32, 4)]),
                    mybir.AxisListType.X, mybir.AluOpType.add)
                nc.vector.tensor_add(out=z[:], in0=z[:], in1=bias_t[:])
                nc.vector.tensor_add(out=outbuf[:], in0=z[:], in1=resid_tile)
                return outbuf

            g1_own = cp.tile([128, NTILES * 32], dt.float32)
            for tile_j, ev in edge_phase(nch1, tiles1, starts1, stops1,
                                         table1, t_s1i, t_s1g, a_d1, "1"):
                g1t = AP(g1_own[:].tensor,
                         g1_own[:].offset + tile_j * 32,
                         [(g1_own[:].ap[0][0], 128), (1, 32)])
                gt = AP(g_own[:].tensor, g_own[:].offset + tile_j * 32,
                        [(g_own[:].ap[0][0], 128), (1, 32)])
                buf = wp.tile([128, 32], dt.float32, tag="g1b")
                pass2(ev, tile_j, t_b1, gt, buf)
                nc.vector.tensor_copy(out=g1t, in_=buf[:])
                # a_d2 tile: transpose g1 then @ A2d
                g1b = wp.tile([128, 32], dt.bfloat16, tag="g1bf")
                nc.vector.tensor_copy(out=g1b[:], in_=buf[:])
                gtp = pp.tile([32, 128], dt.bfloat16, space="PSUM", tag="gtp", bufs=1)
                nc.tensor.transpose(out=gtp[:], in_=g1b[:],
                                    identity=ident[:])
                gts = wp.tile([32, 128], dt.bfloat16, tag="gts")
                nc.vector.tensor_copy(out=gts[:], in_=gtp[:])
                pad2 = pp.tile([128, 4], dt.float32, space="PSUM", tag="pad2", bufs=1)
                nc.tensor.matmul(pad2[:], lhsT=gts[:], rhs=t_a2d[:],
                                 start=True, stop=True)
                nc.vector.tensor_copy(out=a_d2[:, bass.ts(tile_j, 4)],
                                      in_=pad2[:])

            for tile_j, ev in edge_phase(nch2, tiles2, starts2, stops2,
                                         table2, t_s2i, t_s2g, a_d2, "2"):
                g1t = AP(g1_own[:].tensor, g1_own[:].offset + tile_j * 32,
                         [(g1_own[:].ap[0][0], 128), (1, 32)])
                buf = wp.tile([128, 32], dt.float32, tag="g2b")
                pass2(ev, tile_j, t_b2, g1t, buf)
                nc.sync.dma_start(out=outp[bass.ts(tile_j, 128), :],
                                  in_=buf[:])
            edge_psum.__exit__(None, None, None)
    nc.finalize()
    return nc


# --------------------------------------------------------------------------
# entry point
# --------------------------------------------------------------------------


def kernel(x_gene, x_drug, edge_gg, edge_dg,
           Wg, bg, gg_gamma, gg_beta, Wd, bd, dg_gamma, dg_beta,
           W1, att_s1, att_d1, bias1, W2, att_s2, att_d2, bias2):
    global LAST_RESULT
    _install_patches()
    from concourse.bass_utils import run_bass_kernel_spmd
    import os

    f32 = np.float32
    x_gene = np.asarray(x_gene, f32)
    x_drug = np.asarray(x_drug, f32)

    # edges (+self loops, matching reference)
    loop = np.arange(NG, dtype=np.int64)
    e1s = np.concatenate([np.asarray(edge_gg[0], np.int64), loop])
    e1d = np.concatenate([np.asarray(edge_gg[1], np.int64), loop])
    e2s = np.concatenate([np.asarray(edge_dg[0], np.int64), loop])
    e2d = np.concatenate([np.asarray(edge_dg[1], np.int64), loop])
    s1i, s1g, tiles1, cpt1 = _prep_edges(e1s, e1d)
    s2i, s2g, tiles2, cpt2 = _prep_edges(e2s, e2d)

    def startstop(tiles):
        starts = np.zeros(len(tiles), bool)
        stops = np.zeros(len(tiles), bool)
        starts[0] = True
        starts[1:] = tiles[1:] != tiles[:-1]
        stops[-1] = True
        stops[:-1] = tiles[1:] != tiles[:-1]
        return starts, stops

    st1, sp1 = startstop(tiles1)
    st2, sp2 = startstop(tiles2)

    # weights
    W1 = np.asarray(W1, f32)
    W2 = np.asarray(W2, f32)
    w1comb = _wstack(W1, _fold_att(W1, att_s1), _fold_att(W1, att_d1),
                     np.eye(32, dtype=f32))      # [32, 168]
    w2comb = _wstack(W2, _fold_att(W2, att_s2))  # [32, 132]
    a2d = _fold_att(W2, att_d2).astype(bf16)     # [32, 4]

    npad = RPAD - RN
    relu_bg = np.maximum(np.asarray(bg, f32), 0)
    relu_bd = np.maximum(np.asarray(bd, f32), 0)
    statc = np.stack([npad * relu_bg, npad * relu_bg**2,
                      npad * relu_bd, npad * relu_bd**2], axis=1)
    gamb = np.stack([np.asarray(gg_gamma, f32), np.asarray(gg_beta, f32),
                     np.asarray(dg_gamma, f32), np.asarray(dg_beta, f32)],
                    axis=1)

    nc = _build(len(tiles1), tiles1, st1, sp1, len(tiles2), tiles2, st2, sp2)

    in_maps = []
    for k in range(M):
        xg_s = np.zeros((RPAD, 128), f32)
        xg_s[:RN] = x_gene[k * RN:(k + 1) * RN]
        xd_s = np.zeros((RPAD, 96), f32)
        xd_s[:RN] = x_drug[k * RN:(k + 1) * RN]
        in_maps.append({
            "xg": np.ascontiguousarray(xg_s.T).astype(bf16),
            "xd": np.ascontiguousarray(xd_s.T).astype(bf16),
            "wg": np.asarray(Wg, f32).astype(bf16),
            "wd": np.asarray(Wd, f32).astype(bf16),
            "bgv": np.asarray(bg, f32).reshape(32, 1),
            "bdv": np.asarray(bd, f32).reshape(32, 1),
            "statc": statc.astype(f32),
            "gamb": gamb.astype(f32),
            "w1c": w1comb,
            "w2c": w2comb,
            "a2d": a2d,
            "bias1r": np.broadcast_to(np.asarray(bias1, f32), (128, 32)).copy(),
            "bias2r": np.broadcast_to(np.asarray(bias2, f32), (128, 32)).copy(),
            "iotap": np.broadcast_to(np.arange(128, dtype=f32), (128, 128)).copy(),
            "s1i": np.ascontiguousarray(s1i[k].T),
            "s1g": np.ascontiguousarray(s1g[k].T),
            "s2i": np.ascontiguousarray(s2i[k].T),
            "s2g": np.ascontiguousarray(s2g[k].T),
        })

    trace = bool(os.environ.get("TRNGNN_TRACE"))
    res = run_bass_kernel_spmd(nc, in_maps, core_ids=list(range(M)),
                               trace=trace)
    LAST_RESULT = res

    out = np.empty((NG, 32), f32)
    for k in range(M):
        out[k * RN:(k + 1) * RN] = res.results[k]["out"][:RN]
    return out


# revision 6
# speedup vs baseline: 1.0197x; 1.0197x over previous
"""Trainium2 Bass kernel for nn_Columbina_Model (2-layer hetero GAT).

Strategy: dst-shard gene nodes across 8 cores (12500 each, padded to 12544).
Host sorts each core's edges by destination, groups them into 128-edge chunks
per 128-node destination tile. On device: encoders + BN stats (AllReduce) are
computed per-shard; BN is folded into the GAT linear weights; each core builds
its shard of a gather table [h1|a_s] and tables are AllGathered. The edge
phase gathers 288B table rows by src via indirect DMA, computes attention
weights (leaky-relu + exp), and segment-sums messages into per-node-tile PSUM
accumulators with one-hot matmuls — no indirect scatter needed. Softmax
normalization, head-mean, bias and residual run per node tile.
"""
import json
import sys
import types

import numpy as np
import ml_dtypes

H, C = 4, 32
NG = ND = 100000
M = 8
RN = 12500
RPAD = 12544            # 98 * 128
NTILES = RPAD // 128
NT = RPAD * M           # 100352 table rows
DUMMY = NT              # dummy table row (a_s = -1e9, h = 0)
TROWS = NT + 8          # table allocation rows
ROWW = 144              # table row width (bf16) = 288B; cols 0..128 h, 128..132 a_s
BN_EPS = 1e-5

bf16 = ml_dtypes.bfloat16

LAST_RESULT = None

# --------------------------------------------------------------------------
# runtime patches (this container lacks antenv.axon_hooks; walrus rejects >1
# sync wait per instruction)
# --------------------------------------------------------------------------


def _install_patches():
    if "antenv.axon_hooks" not in sys.modules:
        try:
            import antenv
            from trn_agent_boot.trn_boot import _ntff_profile_via_ctypes

            m = types.ModuleType("antenv.axon_hooks")
            _state = {"hook": _ntff_profile_via_ctypes("/opt/axon/libaxon_pjrt.so")}
            m.get_axon_ntff_profile_hook = lambda: _state["hook"]
            m.set_axon_ntff_profile_hook = lambda h: _state.__setitem__("hook", h)
            sys.modules["antenv.axon_hooks"] = m
            antenv.axon_hooks = m
        except Exception:
            pass

    import concourse.tile as tile
    from concourse.vector_clock import ScopedClock

    if not getattr(tile.TileContext, "_drain_patched", False):
        def _drain_and_barrier(self, tick_clock, wait_clock):
            gc = tick_clock.global_clock
            for proc in range(len(gc)):
                tick = gc[proc]
                if tick > 0:
                    partial = ScopedClock()
                    partial.require_at_least(None, proc, tick)
                    nop = self.nc.sync.nop(nofuse=True)
                    wait_clock.add_sem_waits(nop.ins, partial)
            self.nc.sync.drain()
            self.nc.all_engine_barrier()
            assert self.sems is not None
            popped = self.nc._tile_sem_poison_stack.pop()
            assert popped is self._sem_poison
            self.nc.clear_and_free_semaphores(list(self.sems.allocated().values()))
            self.nc.all_engine_barrier()

        tile.TileContext._drain_and_barrier = _drain_and_barrier
        tile.TileContext._drain_patched = True

    import concourse.bass_utils as bu
    import concourse.bass2jax as b2j

    if not getattr(bu, "_compile_patched", False):
        def _split(bir_json):
            d = json.loads(bir_json)
            for f in d.get("functions", []):
                for bb in f.get("blocks", []):
                    new_insts = []
                    for inst in bb["instructions"]:
                        si = inst.get("sync_info")
                        ow = (si or {}).get("on_wait") or []
                        if len(ow) > 1:
                            for i, w in enumerate(ow[:-1]):
                                new_insts.append({
                                    "debug": inst.get("debug", 0),
                                    "engine": inst["engine"],
                                    "ins": [], "outs": [],
                                    "name": f"{inst['name']}-w{i}",
                                    "opcode": "NoOp",
                                    "sync_info": {"on_update": [],
                                                  "on_wait": [w]},
                                })
                            si["on_wait"] = ow[-1:]
                        new_insts.append(inst)
                    bb["instructions"] = new_insts
            return json.dumps(d).encode()

        orig = bu.compile_bir_kernel

        def compile_bir_kernel(bir_json, tmpdir, neff_name="file.neff"):
            return orig(_split(bir_json), tmpdir, neff_name)

        bu.compile_bir_kernel = compile_bir_kernel
        b2j.compile_bir_kernel = compile_bir_kernel
        bu._compile_patched = True


# --------------------------------------------------------------------------
# host-side edge preprocessing
# --------------------------------------------------------------------------


def _fold_att(W, att):
    # W [32, 128], att [H, C] -> [32, H]
    return np.einsum('jhc,hc->jh', np.asarray(W).reshape(32, H, C), np.asarray(att))


def _remap(v):
    return (v // RN) * RPAD + (v % RN)


def _prep_edges(src, dst):
    """Per-core chunk arrays. Returns (src_idx [M,NCH,128] int32,
    seg [M,NCH,128] f32, tile_of_chunk [NCH] int, chunks_per_tile [NTILES])."""
    src = np.asarray(src).astype(np.int64)
    dst = np.asarray(dst).astype(np.int64)
    per_core = []
    counts = np.zeros((M, NTILES), np.int64)
    for k in range(M):
        m = (dst >= k * RN) & (dst < (k + 1) * RN)
        s, d = src[m], dst[m] - k * RN
        order = np.argsort(d, kind='stable')
        s, d = s[order], d[order]
        tile_id = d >> 7
        counts[k] = np.bincount(tile_id, minlength=NTILES)
        per_core.append((s, d, tile_id))
    cpt = np.maximum(1, (counts.max(axis=0) + 127) // 128)  # chunks per tile
    nch = int(cpt.sum())
    chunk_base = np.concatenate([[0], np.cumsum(cpt)])[:-1]  # per tile
    tile_of_chunk = np.repeat(np.arange(NTILES), cpt)
    src_idx = np.full((M, nch, 128), DUMMY, np.int32)
    seg = np.zeros((M, nch, 128), np.float32)
    for k in range(M):
        s, d, tile_id = per_core[k]
        tile_start = np.concatenate([[0], np.cumsum(counts[k])])[:-1]
        pos = np.arange(len(s)) - tile_start[tile_id]
        gchunk = chunk_base[tile_id] + (pos >> 7)
        slot = pos & 127
        src_idx[k, gchunk, slot] = _remap(s)
        seg[k, gchunk, slot] = (d & 127).astype(np.float32)
    return src_idx, seg, tile_of_chunk, cpt


def _wstack(*mats):
    return np.concatenate([np.asarray(m, np.float32) for m in mats], axis=1)


# --------------------------------------------------------------------------
# device kernel builder
# --------------------------------------------------------------------------


def _build(nch1, tiles1, starts1, stops1, nch2, tiles2, starts2, stops2):
    import concourse.bass as bass
    import concourse.mybir as mybir
    import concourse.tile as tile
    from concourse import bacc
    from concourse.bass import AP, IndirectOffsetOnAxis
    from concourse.masks import make_identity

    dt = mybir.dt
    nc = bacc.Bacc()

    def par(name, shape, dty, out=False):
        return nc.declare_dram_parameter(name, shape, dty, isOutput=out)

    xg = par("xg", [128, RPAD], dt.bfloat16)        # transposed gene shard
    xd = par("xd", [96, RPAD], dt.bfloat16)         # transposed drug shard
    wg = par("wg", [128, 32], dt.bfloat16)
    wd = par("wd", [96, 32], dt.bfloat16)
    bgv = par("bgv", [32, 1], dt.float32)           # encoder biases (per channel)
    bdv = par("bdv", [32, 1], dt.float32)
    statc = par("statc", [32, 4], dt.float32)       # pad-col stat corrections
    gamb = par("gamb", [32, 4], dt.float32)         # gamma_g, beta_g, gamma_d, beta_d
    w1c = par("w1c", [32, 168], dt.float32)         # [W1|A_s1|A_d1|I] combined gene
    w2c = par("w2c", [32, 132], dt.float32)         # [W2|A_s2] drug
    a2d = par("a2d", [32, 4], dt.bfloat16)          # fold(W2, att_d2), unscaled
    bias1r = par("bias1r", [128, 32], dt.float32)   # bias1 replicated
    bias2r = par("bias2r", [128, 32], dt.float32)
    iotap = par("iotap", [128, 128], dt.float32)
    s1i = par("s1i", [128, nch1], dt.int32)
    s1g = par("s1g", [128, nch1], dt.float32)
    s2i = par("s2i", [128, nch2], dt.int32)
    s2g = par("s2g", [128, nch2], dt.float32)
    outp = par("out", [RPAD, 32], dt.float32, out=True)

    with tile.TileContext(nc) as tc:
        with (
            tc.tile_pool(name="const", bufs=1) as cp,
            tc.tile_pool(name="enc", bufs=2) as ep,
            tc.tile_pool(name="work", bufs=4) as wp,
            tc.tile_pool(name="dram", bufs=1, space="DRAM") as dp,
        ):
            # ---------------- constants / params to SBUF ----------------
            t_wg = cp.tile([128, 32], dt.bfloat16)
            t_wd = cp.tile([96, 32], dt.bfloat16)
            t_bg = cp.tile([32, 1], dt.float32)
            t_bd = cp.tile([32, 1], dt.float32)
            t_statc = cp.tile([32, 4], dt.float32)
            t_gamb = cp.tile([32, 4], dt.float32)
            t_w1c = cp.tile([32, 168], dt.float32)
            t_w2c = cp.tile([32, 132], dt.float32)
            t_a2d = cp.tile([32, 4], dt.bfloat16)
            t_b1 = cp.tile([128, 32], dt.float32)
            t_b2 = cp.tile([128, 32], dt.float32)
            t_iota = cp.tile([128, 128], dt.float32)
            t_s1i = cp.tile([128, nch1], dt.int32)
            t_s1g = cp.tile([128, nch1], dt.float32)
            t_s2i = cp.tile([128, nch2], dt.int32)
            t_s2g = cp.tile([128, nch2], dt.float32)
            ident = cp.tile([128, 128], dt.bfloat16)
            make_identity(nc, ident[:])
            for t, p in [(t_wg, wg), (t_wd, wd), (t_bg, bgv), (t_bd, bdv),
                         (t_statc, statc), (t_gamb, gamb), (t_w1c, w1c),
                         (t_w2c, w2c), (t_a2d, a2d), (t_b1, bias1r),
                         (t_b2, bias2r), (t_iota, iotap), (t_s1i, s1i),
                         (t_s1g, s1g), (t_s2i, s2i), (t_s2g, s2g)]:
                nc.sync.dma_start(out=t[:], in_=p[:])
            t_xg = cp.tile([128, RPAD], dt.bfloat16)
            t_xd = cp.tile([96, RPAD], dt.bfloat16)
            nc.sync.dma_start(out=t_xg[:], in_=xg[:])
            nc.sync.dma_start(out=t_xd[:], in_=xd[:])

            pp = None  # set per-phase below
            # DRAM internals
            table1 = dp.tile([TROWS, ROWW], dt.bfloat16)
            table2 = dp.tile([TROWS, ROWW], dt.bfloat16)
            own1 = dp.tile([RPAD, ROWW], dt.bfloat16)
            own2 = dp.tile([RPAD, ROWW], dt.bfloat16)
            st_in = dp.tile([32, 4], dt.float32)
            st_out = dp.tile([32, 4], dt.float32)

            enc_psum = tc.tile_pool(name="ps_enc", bufs=2, space="PSUM")
            pp = enc_psum.__enter__()
            # ---------------- encoders: y_T = relu(W^T x_T + b) ---------
            def encode(xT, wT, bT, npart, tag):
                yT = cp.tile([33, RPAD], dt.bfloat16, tag=f"y{tag}")
                st = ep.tile([32, 50], dt.float32, tag=f"st{tag}")
                for i in range(25):
                    lo = i * 512
                    w = min(512, RPAD - lo)
                    ps = pp.tile([32, 512], dt.float32, space="PSUM", tag="encp")
                    nc.tensor.matmul(ps[:, :w], lhsT=wT[:npart, :],
                                     rhs=xT[:npart, lo:lo + w],
                                     start=True, stop=True)
                    nc.scalar.activation(
                        yT[0:32, lo:lo + w], ps[:, :w],
                        mybir.ActivationFunctionType.Relu, bias=bT[:],
                        accum_out=st[:, i:i + 1])
                    sq = ep.tile([32, 512], dt.float32, tag=f"sq{tag}")
                    nc.scalar.activation(
                        sq[:, :w], yT[0:32, lo:lo + w],
                        mybir.ActivationFunctionType.Square,
                        accum_out=st[:, 25 + i:26 + i])
                nc.vector.memset(yT[32:33, :], 1.0)
                s1 = ep.tile([32, 2], dt.float32, tag=f"s1{tag}")
                nc.vector.tensor_reduce(s1[:, 0:1], st[:, 0:25],
                                        mybir.AxisListType.X, mybir.AluOpType.add)
                nc.vector.tensor_reduce(s1[:, 1:2], st[:, 25:50],
                                        mybir.AxisListType.X, mybir.AluOpType.add)
                return yT, s1

            ygT, sg = encode(t_xg, t_wg, t_bg, 128, "g")
            ydT, sd = encode(t_xd, t_wd, t_bd, 96, "d")

            # stats allreduce: [32,4] = [sum_g, sq_g, sum_d, sq_d] - corrections
            packed = ep.tile([32, 4], dt.float32, tag="pk")
            nc.vector.tensor_copy(out=packed[:, 0:2], in_=sg[:])
            nc.vector.tensor_copy(out=packed[:, 2:4], in_=sd[:])
            nc.vector.tensor_sub(out=packed[:], in0=packed[:], in1=t_statc[:])
            nc.sync.dma_start(out=st_in[:], in_=packed[:])
            nc.gpsimd.collective_compute(
                "AllReduce", mybir.AluOpType.add,
                replica_groups=[list(range(M))],
                ins=[st_in.opt()], outs=[st_out.opt()])
            tstat = ep.tile([32, 4], dt.float32, tag="ts")
            nc.gpsimd.dma_start(out=tstat[:], in_=st_out[:])

            # BN fold: s = gamma/sqrt(var+eps), t = beta - mu*s  (cols: g, d)
            sfold = ep.tile([32, 2], dt.float32, tag="sf")
            tfold = ep.tile([32, 2], dt.float32, tag="tf")
            mu = ep.tile([32, 2], dt.float32, tag="mu")
            var = ep.tile([32, 2], dt.float32, tag="va")
            tmp = ep.tile([32, 2], dt.float32, tag="tm")
            nc.vector.tensor_scalar(out=mu[:, 0:1], in0=tstat[:, 0:1],
                                    scalar1=1.0 / NG, scalar2=None,
                                    op0=mybir.AluOpType.mult)
            nc.vector.tensor_scalar(out=mu[:, 1:2], in0=tstat[:, 2:3],
                                    scalar1=1.0 / ND, scalar2=None,
                                    op0=mybir.AluOpType.mult)
            nc.vector.tensor_scalar(out=var[:, 0:1], in0=tstat[:, 1:2],
                                    scalar1=1.0 / NG, scalar2=None,
                                    op0=mybir.AluOpType.mult)
            nc.vector.tensor_scalar(out=var[:, 1:2], in0=tstat[:, 3:4],
                                    scalar1=1.0 / ND, scalar2=None,
                                    op0=mybir.AluOpType.mult)
            nc.vector.tensor_mul(out=tmp[:], in0=mu[:], in1=mu[:])
            nc.vector.tensor_sub(out=var[:], in0=var[:], in1=tmp[:])
            nc.vector.tensor_scalar(out=var[:], in0=var[:], scalar1=BN_EPS,
                                    scalar2=None, op0=mybir.AluOpType.add)
            nc.scalar.activation(tmp[:], var[:], mybir.ActivationFunctionType.Sqrt)
            nc.vector.reciprocal(out=tmp[:], in_=tmp[:])
            nc.vector.tensor_mul(out=sfold[:, 0:1], in0=t_gamb[:, 0:1],
                                 in1=tmp[:, 0:1])
            nc.vector.tensor_mul(out=sfold[:, 1:2], in0=t_gamb[:, 2:3],
                                 in1=tmp[:, 1:2])
            nc.vector.tensor_mul(out=tfold[:], in0=mu[:], in1=sfold[:])
            nc.vector.tensor_sub(out=tfold[:, 0:1], in0=t_gamb[:, 1:2],
                                 in1=tfold[:, 0:1])
            nc.vector.tensor_sub(out=tfold[:, 1:2], in0=t_gamb[:, 3:4],
                                 in1=tfold[:, 1:2])

            # W augmentation.
            # gene: waug1 [33, 168] = [s_g*[W1|A_s1|A_d1|I] ; t_g@[...]]
            #   cols 0..132 table row, 132..136 a_d1, 136..168 g rows
            def build_waug(wcomb, ncols, sf_col, tf_col, tag):
                wa = cp.tile([33, ncols], dt.bfloat16, tag=f"wa{tag}")
                scaled = ep.tile([32, ncols], dt.float32, tag=f"sc{tag}")
                nc.vector.tensor_tensor(
                    out=scaled[:], in0=wcomb[:, :ncols],
                    in1=sf_col.to_broadcast([32, ncols]),
                    op=mybir.AluOpType.mult)
                nc.vector.tensor_copy(out=wa[0:32, :], in_=scaled[:])
                tr = pp.tile([1, ncols], dt.float32, space="PSUM", tag="warow")
                tscaled = ep.tile([32, 1], dt.float32, tag=f"tsc{tag}")
                nc.vector.tensor_copy(out=tscaled[:], in_=tf_col)
                nc.tensor.matmul(tr[:], lhsT=tscaled[:], rhs=wcomb[:, :ncols],
                                 start=True, stop=True)
                nc.vector.tensor_copy(out=wa[32:33, :], in_=tr[:])
                return wa

            waug1 = build_waug(t_w1c, 168, sfold[:, 0:1], tfold[:, 0:1], "1")
            waug2 = build_waug(t_w2c, 132, sfold[:, 1:2], tfold[:, 1:2], "2")

            # ---------------- own table builds ----------------
            a_d1 = cp.tile([128, NTILES * 4], dt.bfloat16)
            a_d2 = cp.tile([128, NTILES * 4], dt.bfloat16)
            g_own = cp.tile([128, NTILES * 32], dt.float32)

            for j in range(NTILES):
                ps = pp.tile([128, 168], dt.float32, space="PSUM", tag="tb")
                nc.tensor.matmul(ps[:], lhsT=ygT[:, bass.ts(j, 128)],
                                 rhs=waug1[:], start=True, stop=True)
                row = wp.tile([128, ROWW], dt.bfloat16, tag="tbr")
                nc.vector.tensor_copy(out=row[:, 0:132], in_=ps[:, 0:132])
                nc.vector.memset(row[:, 132:ROWW], 0.0)
                nc.sync.dma_start(out=own1[bass.ts(j, 128), :], in_=row[:])
                nc.vector.tensor_copy(out=a_d1[:, bass.ts(j, 4)],
                                      in_=ps[:, 132:136])
                nc.vector.tensor_copy(out=g_own[:, bass.ts(j, 32)],
                                      in_=ps[:, 136:168])
            for j in range(NTILES):
                ps = pp.tile([128, 132], dt.float32, space="PSUM", tag="tb2")
                nc.tensor.matmul(ps[:], lhsT=ydT[:, bass.ts(j, 128)],
                                 rhs=waug2[:], start=True, stop=True)
                row = wp.tile([128, ROWW], dt.bfloat16, tag="tbr2")
                nc.vector.tensor_copy(out=row[:, 0:132], in_=ps[:])
                nc.vector.memset(row[:, 132:ROWW], 0.0)
                nc.sync.dma_start(out=own2[bass.ts(j, 128), :], in_=row[:])

            # dummy rows (a_s = -1e9 so exp underflows to 0)
            drow = wp.tile([8, ROWW], dt.bfloat16, tag="drow")
            nc.vector.memset(drow[:], 0.0)
            nc.vector.memset(drow[:, 128:132], -1e9)
            nc.sync.dma_start(out=table1[NT:TROWS, :], in_=drow[:])
            nc.sync.dma_start(out=table2[NT:TROWS, :], in_=drow[:])

            enc_psum.__exit__(None, None, None)
            edge_psum = tc.tile_pool(name="ps_edge", bufs=2, space="PSUM")
            pp = edge_psum.__enter__()
            # AllGather tables
            nc.gpsimd.collective_compute(
                "AllGather", mybir.AluOpType.bypass,
                replica_groups=[list(range(M))],
                ins=[own1.opt()], outs=[table1[0:NT, :]])
            nc.gpsimd.collective_compute(
                "AllGather", mybir.AluOpType.bypass,
                replica_groups=[list(range(M))],
                ins=[own2.opt()], outs=[table2[0:NT, :]])

            # ---------------- edge phase ----------------
            def apv(t, part, dims, offset=0):
                a = t[:]
                return AP(a.tensor, a.offset + offset,
                          [(a.ap[0][0], part)] + list(dims))

            def edge_phase(nch, tiles, starts, stops, table, t_si, t_sg, a_d,
                           tag):
                """4-chunk-batched edge pipeline; yields (tile, evac) per
                destination node tile."""
                ps = None
                for base in range(0, nch, 4):
                    G = min(4, nch - base)
                    g4 = wp.tile([128, 4, ROWW], dt.bfloat16, tag=f"g{tag}")
                    for i in range(G):
                        nc.gpsimd.indirect_dma_start(
                            out=g4[:, i, :], out_offset=None, in_=table[:],
                            in_offset=IndirectOffsetOnAxis(
                                ap=t_si[:, base + i:base + i + 1], axis=0))
                    P4 = wp.tile([128, 4, 128], dt.bfloat16, tag=f"P{tag}")
                    pst = t_sg[:].ap[0][0]
                    nc.vector.tensor_tensor(
                        out=P4[:, :G, :],
                        in0=AP(t_sg[:].tensor, t_sg[:].offset + base,
                               [(pst, 128), (1, G), (0, 128)]),
                        in1=apv(t_iota, 128, [(0, G), (1, 128)]),
                        op=mybir.AluOpType.is_equal)
                    ptp4 = pp.tile([128, 4, 128], dt.bfloat16, space="PSUM",
                                   tag="ptp", bufs=1)
                    for i in range(G):
                        nc.tensor.transpose(out=ptp4[:, i, :], in_=P4[:, i, :],
                                            identity=ident[:])
                    PT4 = wp.tile([128, 4, 128], dt.bfloat16, tag=f"PT{tag}")
                    nc.vector.tensor_copy(out=apv(PT4, 128, [(1, G * 128)]),
                                          in_=apv(ptp4, 128, [(1, G * 128)]))
                    pa4 = pp.tile([128, 4, 4], dt.float32, space="PSUM",
                                  tag="pa", bufs=1)
                    for i in range(G):
                        nc.tensor.matmul(
                            pa4[:, i, :], lhsT=PT4[:, i, :],
                            rhs=a_d[:, bass.ts(int(tiles[base + i]), 4)],
                            start=True, stop=True)
                    alpha4 = wp.tile([128, 4, 4], dt.float32, tag=f"al{tag}")
                    nc.vector.tensor_tensor(
                        out=alpha4[:, :G, :],
                        in0=apv(g4, 128, [(ROWW, G), (1, 4)], offset=128),
                        in1=apv(pa4, 128, [(4, G), (1, 4)]),
                        op=mybir.AluOpType.add)
                    lr4 = wp.tile([128, 4, 4], dt.float32, tag=f"lr{tag}")
                    nc.vector.scalar_tensor_tensor(
                        out=apv(lr4, 128, [(1, G * 4)]),
                        in0=apv(alpha4, 128, [(1, G * 4)]), scalar=0.2,
                        in1=apv(alpha4, 128, [(1, G * 4)]),
                        op0=mybir.AluOpType.mult, op1=mybir.AluOpType.max)
                    rhs4 = wp.tile([128, 4, 132], dt.bfloat16, tag=f"rh{tag}")
                    nc.scalar.activation(
                        apv(rhs4, 128, [(132, G), (1, 4)], offset=128),
                        apv(lr4, 128, [(4, G), (1, 4)]),
                        mybir.ActivationFunctionType.Exp)
                    nc.vector.tensor_tensor(
                        out=apv(rhs4, 128, [(132, G), (32, 4), (1, 32)]),
                        in0=apv(g4, 128, [(ROWW, G), (32, 4), (1, 32)]),
                        in1=apv(rhs4, 128, [(132, G), (1, 4), (0, 32)],
                                offset=128),
                        op=mybir.AluOpType.mult)
                    for i in range(G):
                        c = base + i
                        if starts[c]:
                            ps = pp.tile([128, 132], dt.float32, space="PSUM",
                                         tag="ac", bufs=2)
                        nc.tensor.matmul(ps[:], lhsT=P4[:, i, :],
                                         rhs=rhs4[:, i, :],
                                         start=bool(starts[c]),
                                         stop=bool(stops[c]))
                        if stops[c]:
                            ev = wp.tile([128, 132], dt.float32, tag=f"ev{tag}")
                            nc.vector.tensor_copy(out=ev[:], in_=ps[:])
                            yield int(tiles[c]), ev

            def pass2(ev, tile_j, bias_t, resid_tile, outbuf):
                # z = mean_h(num_h / (den_h + eps)) + bias + resid
                den = wp.tile([128, 4], dt.float32, tag="den")
                nc.vector.tensor_scalar(out=den[:], in0=ev[:, 128:132],
                                        scalar1=1e-16, scalar2=None,
                                        op0=mybir.AluOpType.add)
                nc.vector.reciprocal(out=den[:], in_=den[:])
                nc.vector.tensor_scalar(out=den[:], in0=den[:],
                                        scalar1=1.0 / H, scalar2=None,
                                        op0=mybir.AluOpType.mult)
                num = wp.tile([128, 128], dt.float32, tag="num")
                nc.vector.tensor_tensor(
                    out=apv(num, 128, [(32, 4), (1, 32)]),
                    in0=apv(ev, 128, [(32, 4), (1, 32)]),
                    in1=apv(den, 128, [(1, 4), (0, 32)]),
                    op=mybir.AluOpType.mult)
                z = wp.tile([128, 32], dt.float32, tag="z")
                nc.vector.tensor_reduce(
                    z[:], apv(num, 128, [(1, 32), (32, 4)]),
                    mybir.AxisListType.X, mybir.AluOpType.add)
                nc.vector.tensor_add(out=z[:], in0=z[:], in1=bias_t[:])
                nc.vector.tensor_add(out=outbuf[:], in0=z[:], in1=resid_tile)
                return outbuf

            g1_own = cp.tile([128, NTILES * 32], dt.float32)
            for tile_j, ev in edge_phase(nch1, tiles1, starts1, stops1,
                                         table1, t_s1i, t_s1g, a_d1, "1"):
                g1t = AP(g1_own[:].tensor,
                         g1_own[:].offset + tile_j * 32,
                         [(g1_own[:].ap[0][0], 128), (1, 32)])
                gt = AP(g_own[:].tensor, g_own[:].offset + tile_j * 32,
                        [(g_own[:].ap[0][0], 128), (1, 32)])
                buf = wp.tile([128, 32], dt.float32, tag="g1b")
                pass2(ev, tile_j, t_b1, gt, buf)
                nc.vector.tensor_copy(out=g1t, in_=buf[:])
                # a_d2 tile: transpose g1 then @ A2d
                g1b = wp.tile([128, 32], dt.bfloat16, tag="g1bf")
                nc.vector.tensor_copy(out=g1b[:], in_=buf[:])
                gtp = pp.tile([32, 128], dt.bfloat16, space="PSUM", tag="gtp", bufs=1)
                nc.tensor.transpose(out=gtp[:], in_=g1b[:],
                                    identity=ident[:])
                gts = wp.tile([32, 128], dt.bfloat16, tag="gts")
                nc.vector.tensor_copy(out=gts[:], in_=gtp[:])
                pad2 = pp.tile([128, 4], dt.float32, space="PSUM", tag="pad2", bufs=1)
                nc.tensor.matmul(pad2[:], lhsT=gts[:], rhs=t_a2d[:],
                                 start=True, stop=True)
                nc.vector.tensor_copy(out=a_d2[:, bass.ts(tile_j, 4)],
                                      in_=pad2[:])

            for tile_j, ev in edge_phase(nch2, tiles2, starts2, stops2,
                                         table2, t_s2i, t_s2g, a_d2, "2"):
                g1t = AP(g1_own[:].tensor, g1_own[:].offset + tile_j * 32,
                         [(g1_own[:].ap[0][0], 128), (1, 32)])
                buf = wp.tile([128, 32], dt.float32, tag="g2b")
                pass2(ev, tile_j, t_b2, g1t, buf)
                nc.sync.dma_start(out=outp[bass.ts(tile_j, 128), :],
                                  in_=buf[:])
            edge_psum.__exit__(None, None, None)
    nc.finalize()
    return nc


# --------------------------------------------------------------------------
# entry point
# --------------------------------------------------------------------------


def kernel(x_gene, x_drug, edge_gg, edge_dg,
           Wg, bg, gg_gamma, gg_beta, Wd, bd, dg_gamma, dg_beta,
           W1, att_s1, att_d1, bias1, W2, att_s2, att_d2, bias2):
    global LAST_RESULT
    _install_patches()
    from concourse.bass_utils import run_bass_kernel_spmd
    import os

    f32 = np.float32
    x_gene = np.asarray(x_gene, f32)
    x_drug = np.asarray(x_drug, f32)

    # edges (+self loops, matching reference)
    loop = np.arange(NG, dtype=np.int64)
    e1s = np.concatenate([np.asarray(edge_gg[0], np.int64), loop])
    e1d = np.concatenate([np.asarray(edge_gg[1], np.int64), loop])
    e2s = np.concatenate([np.asarray(edge_dg[0], np.int64), loop])
    e2d = np.concatenate([np.asarray(edge_dg[1], np.int64), loop])
    s1i, s1g, tiles1, cpt1 = _prep_edges(e1s, e1d)
    s2i, s2g, tiles2, cpt2 = _prep_edges(e2s, e2d)

    def startstop(tiles):
        starts = np.zeros(len(tiles), bool)
        stops = np.zeros(len(tiles), bool)
        starts[0] = True
        starts[1:] = tiles[1:] != tiles[:-1]
        stops[-1] = True
        stops[:-1] = tiles[1:] != tiles[:-1]
        return starts, stops

    st1, sp1 = startstop(tiles1)
    st2, sp2 = startstop(tiles2)

    # weights
    W1 = np.asarray(W1, f32)
    W2 = np.asarray(W2, f32)
    w1comb = _wstack(W1, _fold_att(W1, att_s1), _fold_att(W1, att_d1),
                     np.eye(32, dtype=f32))      # [32, 168]
    w2comb = _wstack(W2, _fold_att(W2, att_s2))  # [32, 132]
    a2d = _fold_att(W2, att_d2).astype(bf16)     # [32, 4]

    npad = RPAD - RN
    relu_bg = np.maximum(np.asarray(bg, f32), 0)
    relu_bd = np.maximum(np.asarray(bd, f32), 0)
    statc = np.stack([npad * relu_bg, npad * relu_bg**2,
                      npad * relu_bd, npad * relu_bd**2], axis=1)
    gamb = np.stack([np.asarray(gg_gamma, f32), np.asarray(gg_beta, f32),
                     np.asarray(dg_gamma, f32), np.asarray(dg_beta, f32)],
                    axis=1)

    nc = _build(len(tiles1), tiles1, st1, sp1, len(tiles2), tiles2, st2, sp2)

    in_maps = []
    for k in range(M):
        xg_s = np.zeros((RPAD, 128), f32)
        xg_s[:RN] = x_gene[k * RN:(k + 1) * RN]
        xd_s = np.zeros((RPAD, 96), f32)
        xd_s[:RN] = x_drug[k * RN:(k + 1) * RN]
        in_maps.append({
            "xg": np.ascontiguousarray(xg_s.T).astype(bf16),
            "xd": np.ascontiguousarray(xd_s.T).astype(bf16),
            "wg": np.asarray(Wg, f32).astype(bf16),
            "wd": np.asarray(Wd, f32).astype(bf16),
            "bgv": np.asarray(bg, f32).reshape(32, 1),
            "bdv": np.asarray(bd, f32).reshape(32, 1),
            "statc": statc.astype(f32),
            "gamb": gamb.astype(f32),
            "w1c": w1comb,
            "w2c": w2comb,
            "a2d": a2d,
            "bias1r": np.broadcast_to(np.asarray(bias1, f32), (128, 32)).copy(),
            "bias2r": np.broadcast_to(np.asarray(bias2, f32), (128, 32)).copy(),
            "iotap": np.broadcast_to(np.arange(128, dtype=f32), (128, 128)).copy(),
            "s1i": np.ascontiguousarray(s1i[k].T),
            "s1g": np.ascontiguousarray(s1g[k].T),
            "s2i": np.ascontiguousarray(s2i[k].T),
            "s2g": np.ascontiguousarray(s2g[k].T),
        })

    trace = bool(os.environ.get("TRNGNN_TRACE"))
    res = run_bass_kernel_spmd(nc, in_maps, core_ids=list(range(M)),
                               trace=trace)
    LAST_RESULT = res

    out = np.empty((NG, 32), f32)
    for k in range(M):
        out[k * RN:(k + 1) * RN] = res.results[k]["out"][:RN]
    return out


# revision 8
# speedup vs baseline: 1.0272x; 1.0073x over previous
"""Trainium2 Bass kernel for nn_Columbina_Model (2-layer hetero GAT).

Strategy: dst-shard gene nodes across 8 cores (12500 each, padded to 12544).
Host sorts each core's edges by destination, groups them into 128-edge chunks
per 128-node destination tile. On device: encoders + BN stats (AllReduce) are
computed per-shard; BN is folded into the GAT linear weights; each core builds
its shard of a gather table [h1|a_s] and tables are AllGathered. The edge
phase gathers 288B table rows by src via indirect DMA, computes attention
weights (leaky-relu + exp), and segment-sums messages into per-node-tile PSUM
accumulators with one-hot matmuls — no indirect scatter needed. Softmax
normalization, head-mean, bias and residual run per node tile.
"""
import json
import sys
import types

import numpy as np
import ml_dtypes

H, C = 4, 32
NG = ND = 100000
M = 8
RN = 12500
RPAD = 12544            # 98 * 128
NTILES = RPAD // 128
NT = RPAD * M           # 100352 table rows
DUMMY = NT              # dummy table row (a_s = -1e9, h = 0)
TROWS = NT + 8          # table allocation rows
ROWW = 144              # table row width (bf16) = 288B; cols 0..128 h, 128..132 a_s
BN_EPS = 1e-5

bf16 = ml_dtypes.bfloat16

LAST_RESULT = None

# --------------------------------------------------------------------------
# runtime patches (this container lacks antenv.axon_hooks; walrus rejects >1
# sync wait per instruction)
# --------------------------------------------------------------------------


def _install_patches():
    if "antenv.axon_hooks" not in sys.modules:
        try:
            import antenv
            from trn_agent_boot.trn_boot import _ntff_profile_via_ctypes

            m = types.ModuleType("antenv.axon_hooks")
            _state = {"hook": _ntff_profile_via_ctypes("/opt/axon/libaxon_pjrt.so")}
            m.get_axon_ntff_profile_hook = lambda: _state["hook"]
            m.set_axon_ntff_profile_hook = lambda h: _state.__setitem__("hook", h)
            sys.modules["antenv.axon_hooks"] = m
            antenv.axon_hooks = m
        except Exception:
            pass

    import concourse.tile as tile
    from concourse.vector_clock import ScopedClock

    if not getattr(tile.TileContext, "_drain_patched", False):
        def _drain_and_barrier(self, tick_clock, wait_clock):
            gc = tick_clock.global_clock
            for proc in range(len(gc)):
                tick = gc[proc]
                if tick > 0:
                    partial = ScopedClock()
                    partial.require_at_least(None, proc, tick)
                    nop = self.nc.sync.nop(nofuse=True)
                    wait_clock.add_sem_waits(nop.ins, partial)
            self.nc.sync.drain()
            self.nc.all_engine_barrier()
            assert self.sems is not None
            popped = self.nc._tile_sem_poison_stack.pop()
            assert popped is self._sem_poison
            self.nc.clear_and_free_semaphores(list(self.sems.allocated().values()))
            self.nc.all_engine_barrier()

        tile.TileContext._drain_and_barrier = _drain_and_barrier
        tile.TileContext._drain_patched = True

    import concourse.bass_utils as bu
    import concourse.bass2jax as b2j

    if not getattr(bu, "_compile_patched", False):
        def _split(bir_json):
            d = json.loads(bir_json)
            for f in d.get("functions", []):
                for bb in f.get("blocks", []):
                    new_insts = []
                    for inst in bb["instructions"]:
                        si = inst.get("sync_info")
                        ow = (si or {}).get("on_wait") or []
                        if len(ow) > 1:
                            for i, w in enumerate(ow[:-1]):
                                new_insts.append({
                                    "debug": inst.get("debug", 0),
                                    "engine": inst["engine"],
                                    "ins": [], "outs": [],
                                    "name": f"{inst['name']}-w{i}",
                                    "opcode": "NoOp",
                                    "sync_info": {"on_update": [],
                                                  "on_wait": [w]},
                                })
                            si["on_wait"] = ow[-1:]
                        new_insts.append(inst)
                    bb["instructions"] = new_insts
            return json.dumps(d).encode()

        orig = bu.compile_bir_kernel

        def compile_bir_kernel(bir_json, tmpdir, neff_name="file.neff"):
            return orig(_split(bir_json), tmpdir, neff_name)

        bu.compile_bir_kernel = compile_bir_kernel
        b2j.compile_bir_kernel = compile_bir_kernel
        bu._compile_patched = True


# --------------------------------------------------------------------------
# host-side edge preprocessing
# --------------------------------------------------------------------------


def _fold_att(W, att):
    # W [32, 128], att [H, C] -> [32, H]
    return np.einsum('jhc,hc->jh', np.asarray(W).reshape(32, H, C), np.asarray(att))


def _remap(v):
    return (v // RN) * RPAD + (v % RN)


def _prep_edges(src, dst):
    """Per-core chunk arrays. Returns (src_idx [M,NCH,128] int32,
    seg [M,NCH,128] f32, tile_of_chunk [NCH] int, chunks_per_tile [NTILES])."""
    src = np.asarray(src).astype(np.int64)
    dst = np.asarray(dst).astype(np.int64)
    per_core = []
    counts = np.zeros((M, NTILES), np.int64)
    for k in range(M):
        m = (dst >= k * RN) & (dst < (k + 1) * RN)
        s, d = src[m], dst[m] - k * RN
        order = np.argsort(d, kind='stable')
        s, d = s[order], d[order]
        tile_id = d >> 7
        counts[k] = np.bincount(tile_id, minlength=NTILES)
        per_core.append((s, d, tile_id))
    cpt = np.maximum(1, (counts.max(axis=0) + 127) // 128)  # chunks per tile
    nch = int(cpt.sum())
    chunk_base = np.concatenate([[0], np.cumsum(cpt)])[:-1]  # per tile
    tile_of_chunk = np.repeat(np.arange(NTILES), cpt)
    src_idx = np.full((M, nch, 128), DUMMY, np.int32)
    seg = np.zeros((M, nch, 128), np.float32)
    for k in range(M):
        s, d, tile_id = per_core[k]
        tile_start = np.concatenate([[0], np.cumsum(counts[k])])[:-1]
        pos = np.arange(len(s)) - tile_start[tile_id]
        gchunk = chunk_base[tile_id] + (pos >> 7)
        slot = pos & 127
        src_idx[k, gchunk, slot] = _remap(s)
        seg[k, gchunk, slot] = (d & 127).astype(np.float32)
    return src_idx, seg, tile_of_chunk, cpt


def _wstack(*mats):
    return np.concatenate([np.asarray(m, np.float32) for m in mats], axis=1)


# --------------------------------------------------------------------------
# device kernel builder
# --------------------------------------------------------------------------


def _build(nch1, tiles1, starts1, stops1, nch2, tiles2, starts2, stops2):
    import concourse.bass as bass
    import concourse.mybir as mybir
    import concourse.tile as tile
    from concourse import bacc
    from concourse.bass import AP, IndirectOffsetOnAxis
    from concourse.masks import make_identity

    dt = mybir.dt
    nc = bacc.Bacc()

    def par(name, shape, dty, out=False):
        return nc.declare_dram_parameter(name, shape, dty, isOutput=out)

    xg = par("xg", [128, RPAD], dt.bfloat16)        # transposed gene shard
    xd = par("xd", [96, RPAD], dt.bfloat16)         # transposed drug shard
    wg = par("wg", [128, 32], dt.bfloat16)
    wd = par("wd", [96, 32], dt.bfloat16)
    bgv = par("bgv", [32, 1], dt.float32)           # encoder biases (per channel)
    bdv = par("bdv", [32, 1], dt.float32)
    statc = par("statc", [32, 4], dt.float32)       # pad-col stat corrections
    gamb = par("gamb", [32, 4], dt.float32)         # gamma_g, beta_g, gamma_d, beta_d
    w1c = par("w1c", [32, 168], dt.float32)         # [W1|A_s1|A_d1|I] combined gene
    w2c = par("w2c", [32, 132], dt.float32)         # [W2|A_s2] drug
    a2d = par("a2d", [32, 4], dt.bfloat16)          # fold(W2, att_d2), unscaled
    bias1r = par("bias1r", [128, 32], dt.float32)   # bias1 replicated
    bias2r = par("bias2r", [128, 32], dt.float32)
    iotap = par("iotap", [128, 128], dt.float32)
    s1i = par("s1i", [128, nch1], dt.int32)
    s1g = par("s1g", [128, nch1], dt.float32)
    s2i = par("s2i", [128, nch2], dt.int32)
    s2g = par("s2g", [128, nch2], dt.float32)
    outp = par("out", [RPAD, 32], dt.float32, out=True)

    with tile.TileContext(nc) as tc:
        with (
            tc.tile_pool(name="const", bufs=1) as cp,
            tc.tile_pool(name="enc", bufs=2) as ep,
            tc.tile_pool(name="work", bufs=4) as wp,
            tc.tile_pool(name="dram", bufs=1, space="DRAM") as dp,
        ):
            # ---------------- constants / params to SBUF ----------------
            t_wg = cp.tile([128, 32], dt.bfloat16)
            t_wd = cp.tile([96, 32], dt.bfloat16)
            t_bg = cp.tile([32, 1], dt.float32)
            t_bd = cp.tile([32, 1], dt.float32)
            t_statc = cp.tile([32, 4], dt.float32)
            t_gamb = cp.tile([32, 4], dt.float32)
            t_w1c = cp.tile([32, 168], dt.float32)
            t_w2c = cp.tile([32, 132], dt.float32)
            t_a2d = cp.tile([32, 4], dt.bfloat16)
            t_b1 = cp.tile([128, 32], dt.float32)
            t_b2 = cp.tile([128, 32], dt.float32)
            t_iota = cp.tile([128, 128], dt.float32)
            t_s1i = cp.tile([128, nch1], dt.int32)
            t_s1g = cp.tile([128, nch1], dt.float32)
            t_s2i = cp.tile([128, nch2], dt.int32)
            t_s2g = cp.tile([128, nch2], dt.float32)
            ident = cp.tile([128, 128], dt.bfloat16)
            make_identity(nc, ident[:])
            for t, p in [(t_wg, wg), (t_wd, wd), (t_bg, bgv), (t_bd, bdv),
                         (t_statc, statc), (t_gamb, gamb), (t_w1c, w1c),
                         (t_w2c, w2c), (t_a2d, a2d), (t_b1, bias1r),
                         (t_b2, bias2r), (t_iota, iotap), (t_s1i, s1i),
                         (t_s1g, s1g), (t_s2i, s2i), (t_s2g, s2g)]:
                nc.sync.dma_start(out=t[:], in_=p[:])
            t_xg = cp.tile([128, RPAD], dt.bfloat16)
            t_xd = cp.tile([96, RPAD], dt.bfloat16)
            nc.sync.dma_start(out=t_xg[:], in_=xg[:])
            nc.sync.dma_start(out=t_xd[:], in_=xd[:])

            pp = None  # set per-phase below
            # DRAM internals
            table1 = dp.tile([TROWS, ROWW], dt.bfloat16)
            table2 = dp.tile([TROWS, ROWW], dt.bfloat16)
            own1 = dp.tile([RPAD, ROWW], dt.bfloat16)
            own2 = dp.tile([RPAD, ROWW], dt.bfloat16)
            st_in = dp.tile([32, 4], dt.float32)
            st_out = dp.tile([32, 4], dt.float32)

            enc_psum = tc.tile_pool(name="ps_enc", bufs=2, space="PSUM")
            pp = enc_psum.__enter__()
            # ---------------- encoders: y_T = relu(W^T x_T + b) ---------
            def encode(xT, wT, bT, npart, tag):
                yT = cp.tile([33, RPAD], dt.bfloat16, tag=f"y{tag}")
                st = ep.tile([32, 50], dt.float32, tag=f"st{tag}")
                for i in range(25):
                    lo = i * 512
                    w = min(512, RPAD - lo)
                    ps = pp.tile([32, 512], dt.float32, space="PSUM", tag="encp")
                    nc.tensor.matmul(ps[:, :w], lhsT=wT[:npart, :],
                                     rhs=xT[:npart, lo:lo + w],
                                     start=True, stop=True)
                    nc.scalar.activation(
                        yT[0:32, lo:lo + w], ps[:, :w],
                        mybir.ActivationFunctionType.Relu, bias=bT[:],
                        accum_out=st[:, i:i + 1])
                    sq = ep.tile([32, 512], dt.float32, tag=f"sq{tag}")
                    nc.scalar.activation(
                        sq[:, :w], yT[0:32, lo:lo + w],
                        mybir.ActivationFunctionType.Square,
                        accum_out=st[:, 25 + i:26 + i])
                nc.vector.memset(yT[32:33, :], 1.0)
                s1 = ep.tile([32, 2], dt.float32, tag=f"s1{tag}")
                nc.vector.tensor_reduce(s1[:, 0:1], st[:, 0:25],
                                        mybir.AxisListType.X, mybir.AluOpType.add)
                nc.vector.tensor_reduce(s1[:, 1:2], st[:, 25:50],
                                        mybir.AxisListType.X, mybir.AluOpType.add)
                return yT, s1

            ygT, sg = encode(t_xg, t_wg, t_bg, 128, "g")
            ydT, sd = encode(t_xd, t_wd, t_bd, 96, "d")

            # stats allreduce: [32,4] = [sum_g, sq_g, sum_d, sq_d] - corrections
            packed = ep.tile([32, 4], dt.float32, tag="pk")
            nc.vector.tensor_copy(out=packed[:, 0:2], in_=sg[:])
            nc.vector.tensor_copy(out=packed[:, 2:4], in_=sd[:])
            nc.vector.tensor_sub(out=packed[:], in0=packed[:], in1=t_statc[:])
            nc.sync.dma_start(out=st_in[:], in_=packed[:])
            nc.gpsimd.collective_compute(
                "AllReduce", mybir.AluOpType.add,
                replica_groups=[list(range(M))],
                ins=[st_in.opt()], outs=[st_out.opt()])
            tstat = ep.tile([32, 4], dt.float32, tag="ts")
            nc.gpsimd.dma_start(out=tstat[:], in_=st_out[:])

            # BN fold: s = gamma/sqrt(var+eps), t = beta - mu*s  (cols: g, d)
            sfold = ep.tile([32, 2], dt.float32, tag="sf")
            tfold = ep.tile([32, 2], dt.float32, tag="tf")
            mu = ep.tile([32, 2], dt.float32, tag="mu")
            var = ep.tile([32, 2], dt.float32, tag="va")
            tmp = ep.tile([32, 2], dt.float32, tag="tm")
            nc.vector.tensor_scalar(out=mu[:, 0:1], in0=tstat[:, 0:1],
                                    scalar1=1.0 / NG, scalar2=None,
                                    op0=mybir.AluOpType.mult)
            nc.vector.tensor_scalar(out=mu[:, 1:2], in0=tstat[:, 2:3],
                                    scalar1=1.0 / ND, scalar2=None,
                                    op0=mybir.AluOpType.mult)
            nc.vector.tensor_scalar(out=var[:, 0:1], in0=tstat[:, 1:2],
                                    scalar1=1.0 / NG, scalar2=None,
                                    op0=mybir.AluOpType.mult)
            nc.vector.tensor_scalar(out=var[:, 1:2], in0=tstat[:, 3:4],
                                    scalar1=1.0 / ND, scalar2=None,
                                    op0=mybir.AluOpType.mult)
            nc.vector.tensor_mul(out=tmp[:], in0=mu[:], in1=mu[:])
            nc.vector.tensor_sub(out=var[:], in0=var[:], in1=tmp[:])
            nc.vector.tensor_scalar(out=var[:], in0=var[:], scalar1=BN_EPS,
                                    scalar2=None, op0=mybir.AluOpType.add)
            nc.scalar.activation(tmp[:], var[:], mybir.ActivationFunctionType.Sqrt)
            nc.vector.reciprocal(out=tmp[:], in_=tmp[:])
            nc.vector.tensor_mul(out=sfold[:, 0:1], in0=t_gamb[:, 0:1],
                                 in1=tmp[:, 0:1])
            nc.vector.tensor_mul(out=sfold[:, 1:2], in0=t_gamb[:, 2:3],
                                 in1=tmp[:, 1:2])
            nc.vector.tensor_mul(out=tfold[:], in0=mu[:], in1=sfold[:])
            nc.vector.tensor_sub(out=tfold[:, 0:1], in0=t_gamb[:, 1:2],
                                 in1=tfold[:, 0:1])
            nc.vector.tensor_sub(out=tfold[:, 1:2], in0=t_gamb[:, 3:4],
                                 in1=tfold[:, 1:2])

            # W augmentation.
            # gene: waug1 [33, 168] = [s_g*[W1|A_s1|A_d1|I] ; t_g@[...]]
            #   cols 0..132 table row, 132..136 a_d1, 136..168 g rows
            def build_waug(wcomb, ncols, sf_col, tf_col, tag):
                wa = cp.tile([33, ncols], dt.bfloat16, tag=f"wa{tag}")
                scaled = ep.tile([32, ncols], dt.float32, tag=f"sc{tag}")
                nc.vector.tensor_tensor(
                    out=scaled[:], in0=wcomb[:, :ncols],
                    in1=sf_col.to_broadcast([32, ncols]),
                    op=mybir.AluOpType.mult)
                nc.vector.tensor_copy(out=wa[0:32, :], in_=scaled[:])
                tr = pp.tile([1, ncols], dt.float32, space="PSUM", tag="warow")
                tscaled = ep.tile([32, 1], dt.float32, tag=f"tsc{tag}")
                nc.vector.tensor_copy(out=tscaled[:], in_=tf_col)
                nc.tensor.matmul(tr[:], lhsT=tscaled[:], rhs=wcomb[:, :ncols],
                                 start=True, stop=True)
                nc.vector.tensor_copy(out=wa[32:33, :], in_=tr[:])
                return wa

            waug1 = build_waug(t_w1c, 168, sfold[:, 0:1], tfold[:, 0:1], "1")
            waug2 = build_waug(t_w2c, 132, sfold[:, 1:2], tfold[:, 1:2], "2")

            # ---------------- own table builds ----------------
            a_d1 = cp.tile([128, NTILES * 4], dt.bfloat16)
            a_d2 = cp.tile([128, NTILES * 4], dt.bfloat16)
            g_own = cp.tile([128, NTILES * 32], dt.float32)

            for j in range(NTILES):
                ps = pp.tile([128, 168], dt.float32, space="PSUM", tag="tb")
                nc.tensor.matmul(ps[:], lhsT=ygT[:, bass.ts(j, 128)],
                                 rhs=waug1[:], start=True, stop=True)
                row = wp.tile([128, ROWW], dt.bfloat16, tag="tbr")
                nc.vector.tensor_copy(out=row[:, 0:132], in_=ps[:, 0:132])
                nc.vector.memset(row[:, 132:ROWW], 0.0)
                nc.sync.dma_start(out=own1[bass.ts(j, 128), :], in_=row[:])
                nc.vector.tensor_copy(out=a_d1[:, bass.ts(j, 4)],
                                      in_=ps[:, 132:136])
                nc.vector.tensor_copy(out=g_own[:, bass.ts(j, 32)],
                                      in_=ps[:, 136:168])
            for j in range(NTILES):
                ps = pp.tile([128, 132], dt.float32, space="PSUM", tag="tb2")
                nc.tensor.matmul(ps[:], lhsT=ydT[:, bass.ts(j, 128)],
                                 rhs=waug2[:], start=True, stop=True)
                row = wp.tile([128, ROWW], dt.bfloat16, tag="tbr2")
                nc.vector.tensor_copy(out=row[:, 0:132], in_=ps[:])
                nc.vector.memset(row[:, 132:ROWW], 0.0)
                nc.sync.dma_start(out=own2[bass.ts(j, 128), :], in_=row[:])

            # dummy rows (a_s = -1e9 so exp underflows to 0)
            drow = wp.tile([8, ROWW], dt.bfloat16, tag="drow")
            nc.vector.memset(drow[:], 0.0)
            nc.vector.memset(drow[:, 128:132], -1e9)
            nc.sync.dma_start(out=table1[NT:TROWS, :], in_=drow[:])
            nc.sync.dma_start(out=table2[NT:TROWS, :], in_=drow[:])

            enc_psum.__exit__(None, None, None)
            edge_psum = tc.tile_pool(name="ps_edge", bufs=2, space="PSUM")
            pp = edge_psum.__enter__()
            # AllGather tables
            nc.gpsimd.collective_compute(
                "AllGather", mybir.AluOpType.bypass,
                replica_groups=[list(range(M))],
                ins=[own1.opt()], outs=[table1[0:NT, :]])
            nc.gpsimd.collective_compute(
                "AllGather", mybir.AluOpType.bypass,
                replica_groups=[list(range(M))],
                ins=[own2.opt()], outs=[table2[0:NT, :]])

            # ---------------- edge phase ----------------
            def apv(t, part, dims, offset=0):
                a = t[:]
                return AP(a.tensor, a.offset + offset,
                          [(a.ap[0][0], part)] + list(dims))

            def edge_phase(nch, tiles, starts, stops, table, t_si, t_sg, a_d,
                           tag):
                """4-chunk-batched edge pipeline; yields (tile, evac) per
                destination node tile."""
                ps = None
                for base in range(0, nch, 4):
                    G = min(4, nch - base)
                    g4 = wp.tile([128, 4, ROWW], dt.bfloat16, tag="eg", bufs=8)
                    for i in range(G):
                        nc.gpsimd.indirect_dma_start(
                            out=g4[:, i, :], out_offset=None, in_=table[:],
                            in_offset=IndirectOffsetOnAxis(
                                ap=t_si[:, base + i:base + i + 1], axis=0))
                    P4 = wp.tile([128, 4, 128], dt.bfloat16, tag="eP", bufs=8)
                    pst = t_sg[:].ap[0][0]
                    nc.vector.tensor_tensor(
                        out=P4[:, :G, :],
                        in0=AP(t_sg[:].tensor, t_sg[:].offset + base,
                               [(pst, 128), (1, G), (0, 128)]),
                        in1=apv(t_iota, 128, [(0, G), (1, 128)]),
                        op=mybir.AluOpType.is_equal)
                    ptp4 = pp.tile([128, 4, 128], dt.bfloat16, space="PSUM",
                                   tag="ptp", bufs=1)
                    for i in range(G):
                        nc.tensor.transpose(out=ptp4[:, i, :], in_=P4[:, i, :],
                                            identity=ident[:])
                    PT4 = wp.tile([128, 4, 128], dt.bfloat16, tag="ePT", bufs=8)
                    nc.vector.tensor_copy(out=apv(PT4, 128, [(1, G * 128)]),
                                          in_=apv(ptp4, 128, [(1, G * 128)]))
                    pa4 = pp.tile([128, 4, 4], dt.float32, space="PSUM",
                                  tag="pa", bufs=1)
                    for i in range(G):
                        nc.tensor.matmul(
                            pa4[:, i, :], lhsT=PT4[:, i, :],
                            rhs=a_d[:, bass.ts(int(tiles[base + i]), 4)],
                            start=True, stop=True)
                    alpha4 = wp.tile([128, 4, 4], dt.float32, tag="eal", bufs=8)
                    nc.vector.tensor_tensor(
                        out=alpha4[:, :G, :],
                        in0=apv(g4, 128, [(ROWW, G), (1, 4)], offset=128),
                        in1=apv(pa4, 128, [(4, G), (1, 4)]),
                        op=mybir.AluOpType.add)
                    lr4 = wp.tile([128, 4, 4], dt.float32, tag="elr", bufs=8)
                    nc.vector.scalar_tensor_tensor(
                        out=apv(lr4, 128, [(1, G * 4)]),
                        in0=apv(alpha4, 128, [(1, G * 4)]), scalar=0.2,
                        in1=apv(alpha4, 128, [(1, G * 4)]),
                        op0=mybir.AluOpType.mult, op1=mybir.AluOpType.max)
                    rhs4 = wp.tile([128, 4, 132], dt.bfloat16, tag="erh", bufs=8)
                    nc.scalar.activation(
                        apv(rhs4, 128, [(132, G), (1, 4)], offset=128),
                        apv(lr4, 128, [(4, G), (1, 4)]),
                        mybir.ActivationFunctionType.Exp)
                    nc.vector.tensor_tensor(
                        out=apv(rhs4, 128, [(132, G), (32, 4), (1, 32)]),
                        in0=apv(g4, 128, [(ROWW, G), (32, 4), (1, 32)]),
                        in1=apv(rhs4, 128, [(132, G), (1, 4), (0, 32)],
                                offset=128),
                        op=mybir.AluOpType.mult)
                    for i in range(G):
                        c = base + i
                        if starts[c]:
                            ps = pp.tile([128, 132], dt.float32, space="PSUM",
                                         tag="ac", bufs=3)
                        nc.tensor.matmul(ps[:], lhsT=P4[:, i, :],
                                         rhs=rhs4[:, i, :],
                                         start=bool(starts[c]),
                                         stop=bool(stops[c]))
                        if stops[c]:
                            ev = wp.tile([128, 132], dt.float32, tag="eev")
                            nc.vector.tensor_copy(out=ev[:], in_=ps[:])
                            yield int(tiles[c]), ev

            def pass2(ev, tile_j, bias_t, resid_tile, outbuf):
                # z = mean_h(num_h / (den_h + eps)) + bias + resid
                den = wp.tile([128, 4], dt.float32, tag="den")
                nc.vector.tensor_scalar(out=den[:], in0=ev[:, 128:132],
                                        scalar1=1e-16, scalar2=None,
                                        op0=mybir.AluOpType.add)
                nc.vector.reciprocal(out=den[:], in_=den[:])
                nc.vector.tensor_scalar(out=den[:], in0=den[:],
                                        scalar1=1.0 / H, scalar2=None,
                                        op0=mybir.AluOpType.mult)
                num = wp.tile([128, 128], dt.float32, tag="num")
                nc.vector.tensor_tensor(
                    out=apv(num, 128, [(32, 4), (1, 32)]),
                    in0=apv(ev, 128, [(32, 4), (1, 32)]),
                    in1=apv(den, 128, [(1, 4), (0, 32)]),
                    op=mybir.AluOpType.mult)
                z = wp.tile([128, 32], dt.float32, tag="z")
                nc.vector.tensor_reduce(
                    z[:], apv(num, 128, [(1, 32), (32, 4)]),
                    mybir.AxisListType.X, mybir.AluOpType.add)
                nc.vector.tensor_add(out=z[:], in0=z[:], in1=bias_t[:])
                nc.vector.tensor_add(out=outbuf[:], in0=z[:], in1=resid_tile)
                return outbuf

            g1_own = cp.tile([128, NTILES * 32], dt.float32)
            for tile_j, ev in edge_phase(nch1, tiles1, starts1, stops1,
                                         table1, t_s1i, t_s1g, a_d1, "1"):
                g1t = AP(g1_own[:].tensor,
                         g1_own[:].offset + tile_j * 32,
                         [(g1_own[:].ap[0][0], 128), (1, 32)])
                gt = AP(g_own[:].tensor, g_own[:].offset + tile_j * 32,
                        [(g_own[:].ap[0][0], 128), (1, 32)])
                buf = wp.tile([128, 32], dt.float32, tag="g1b")
                pass2(ev, tile_j, t_b1, gt, buf)
                nc.vector.tensor_copy(out=g1t, in_=buf[:])
                # a_d2 tile: transpose g1 then @ A2d
                g1b = wp.tile([128, 32], dt.bfloat16, tag="g1bf")
                nc.vector.tensor_copy(out=g1b[:], in_=buf[:])
                gtp = pp.tile([32, 128], dt.bfloat16, space="PSUM", tag="gtp", bufs=1)
                nc.tensor.transpose(out=gtp[:], in_=g1b[:],
                                    identity=ident[:])
                gts = wp.tile([32, 128], dt.bfloat16, tag="gts")
                nc.vector.tensor_copy(out=gts[:], in_=gtp[:])
                pad2 = pp.tile([128, 4], dt.float32, space="PSUM", tag="pad2", bufs=1)
                nc.tensor.matmul(pad2[:], lhsT=gts[:], rhs=t_a2d[:],
                                 start=True, stop=True)
                nc.vector.tensor_copy(out=a_d2[:, bass.ts(tile_j, 4)],
                                      in_=pad2[:])

            for tile_j, ev in edge_phase(nch2, tiles2, starts2, stops2,
                                         table2, t_s2i, t_s2g, a_d2, "2"):
                g1t = AP(g1_own[:].tensor, g1_own[:].offset + tile_j * 32,
                         [(g1_own[:].ap[0][0], 128), (1, 32)])
                buf = wp.tile([128, 32], dt.float32, tag="g2b")
                pass2(ev, tile_j, t_b2, g1t, buf)
                nc.sync.dma_start(out=outp[bass.ts(tile_j, 128), :],
                                  in_=buf[:])
            edge_psum.__exit__(None, None, None)
    nc.finalize()
    return nc


# --------------------------------------------------------------------------
# entry point
# --------------------------------------------------------------------------


def kernel(x_gene, x_drug, edge_gg, edge_dg,
           Wg, bg, gg_gamma, gg_beta, Wd, bd, dg_gamma, dg_beta,
           W1, att_s1, att_d1, bias1, W2, att_s2, att_d2, bias2):
    global LAST_RESULT
    _install_patches()
    from concourse.bass_utils import run_bass_kernel_spmd
    import os

    f32 = np.float32
    x_gene = np.asarray(x_gene, f32)
    x_drug = np.asarray(x_drug, f32)

    # edges (+self loops, matching reference)
    loop = np.arange(NG, dtype=np.int64)
    e1s = np.concatenate([np.asarray(edge_gg[0], np.int64), loop])
    e1d = np.concatenate([np.asarray(edge_gg[1], np.int64), loop])
    e2s = np.concatenate([np.asarray(edge_dg[0], np.int64), loop])
    e2d = np.concatenate([np.asarray(edge_dg[1], np.int64), loop])
    s1i, s1g, tiles1, cpt1 = _prep_edges(e1s, e1d)
    s2i, s2g, tiles2, cpt2 = _prep_edges(e2s, e2d)

    def startstop(tiles):
        starts = np.zeros(len(tiles), bool)
        stops = np.zeros(len(tiles), bool)
        starts[0] = True
        starts[1:] = tiles[1:] != tiles[:-1]
        stops[-1] = True
        stops[:-1] = tiles[1:] != tiles[:-1]
        return starts, stops

    st1, sp1 = startstop(tiles1)
    st2, sp2 = startstop(tiles2)

    # weights
    W1 = np.asarray(W1, f32)
    W2 = np.asarray(W2, f32)
    w1comb = _wstack(W1, _fold_att(W1, att_s1), _fold_att(W1, att_d1),
                     np.eye(32, dtype=f32))      # [32, 168]
    w2comb = _wstack(W2, _fold_att(W2, att_s2))  # [32, 132]
    a2d = _fold_att(W2, att_d2).astype(bf16)     # [32, 4]

    npad = RPAD - RN
    relu_bg = np.maximum(np.asarray(bg, f32), 0)
    relu_bd = np.maximum(np.asarray(bd, f32), 0)
    statc = np.stack([npad * relu_bg, npad * relu_bg**2,
                      npad * relu_bd, npad * relu_bd**2], axis=1)
    gamb = np.stack([np.asarray(gg_gamma, f32), np.asarray(gg_beta, f32),
                     np.asarray(dg_gamma, f32), np.asarray(dg_beta, f32)],
                    axis=1)

    nc = _build(len(tiles1), tiles1, st1, sp1, len(tiles2), tiles2, st2, sp2)

    in_maps = []
    for k in range(M):
        xg_s = np.zeros((RPAD, 128), f32)
        xg_s[:RN] = x_gene[k * RN:(k + 1) * RN]
        xd_s = np.zeros((RPAD, 96), f32)
        xd_s[:RN] = x_drug[k * RN:(k + 1) * RN]
        in_maps.append({
            "xg": np.ascontiguousarray(xg_s.T).astype(bf16),
            "xd": np.ascontiguousarray(xd_s.T).astype(bf16),
            "wg": np.asarray(Wg, f32).astype(bf16),
            "wd": np.asarray(Wd, f32).astype(bf16),
            "bgv": np.asarray(bg, f32).reshape(32, 1),
            "bdv": np.asarray(bd, f32).reshape(32, 1),
            "statc": statc.astype(f32),
            "gamb": gamb.astype(f32),
            "w1c": w1comb,
            "w2c": w2comb,
            "a2d": a2d,
            "bias1r": np.broadcast_to(np.asarray(bias1, f32), (128, 32)).copy(),
            "bias2r": np.broadcast_to(np.asarray(bias2, f32), (128, 32)).copy(),
            "iotap": np.broadcast_to(np.arange(128, dtype=f32), (128, 128)).copy(),
            "s1i": np.ascontiguousarray(s1i[k].T),
            "s1g": np.ascontiguousarray(s1g[k].T),
            "s2i": np.ascontiguousarray(s2i[k].T),
            "s2g": np.ascontiguousarray(s2g[k].T),
        })

    trace = bool(os.environ.get("TRNGNN_TRACE"))
    res = run_bass_kernel_spmd(nc, in_maps, core_ids=list(range(M)),
                               trace=trace)
    LAST_RESULT = res

    out = np.empty((NG, 32), f32)
    for k in range(M):
        out[k * RN:(k + 1) * RN] = res.results[k]["out"][:RN]
    return out
